# revision 39
# baseline (speedup 1.0000x reference)
"""Grouped-Query Attention (S=2048, NQ=32, NKV=8, D=128, HID=4096) on 8 TRN2 NeuronCores.

Sharding: tensor-parallel over heads. Core c owns KV head c and its G=4
query heads (rows c*512..(c+1)*512 of Wq, c*128..(c+1)*128 of Wk/Wv, and
columns c*512..(c+1)*512 of Wo).  Each core computes a partial output
(row-parallel Wo); the host sums the 8 partials.

All matmuls run in fp16 (1 cycle/row on PE) with fp32 PSUM accumulation.
Schedule (hard-won DMA lessons baked in: per-queue transfers serialize
in order; two queues fight for the ~350GB/s aggregate, so ALL input DMA
goes on ONE queue, the sync queue, interleaved in exact consumption
order — order IS priority; the scalar/Act queue carries no DMA):
  - stage A: projections, streaming x once, chunks processed in order
    3,0,1,2.  Chunk 3 (k/v only — no wq needed) runs FIRST so the 4MB
    of wq streams during it; its x is DMA'd directly into a persistent
    x3 buffer (4-kt quads — the ~650ns/trigger issue rate makes few,
    large startup transfers essential) and reused by the deferred q3
    projection.  Junk matmuls pace chunk 3's burn while the DMA
    pipeline ramps (and keep the HAM clock-gate warm).  Weights are
    host-prepped into the exact SBUF layout (contiguous per-partition
    lines, no small-element DMA penalty); ring x tiles batched 2
    kt-tiles per trigger.  Per-head qT tiles keep cross-head
    dependency tracking apart.
  - A->B boundary: the pacc->pbc PSUM pool transition emits a barrier
    on ALL stage-A copy-outs, so in every chunk the four heads' q
    streams are SKEWED one slot apart and (in the last chunk) each
    head's PSUM copy-out is emitted the moment it stops, with kT on
    Act and vT on DVE (bias deferred into B): the trailing copies
    stagger through the q tail and stage B's first score matmul starts
    ~1us after A's last matmul.  Dummy pbc allocations pin pscore to
    the banks stage A frees first.
  - stage B: scores computed transposed S^T[j,i] = kT-slice.T @ qTh[h],
    exp on Act in [128,1024] tiles (scale folded in, bias -1 for fp16
    range), rowsums accumulated on DVE in fp16, cross-partition sum via
    a ones-matmul, 1/r as exp(-ln(r)) on Act (ln and exp share one
    activation table, explicitly preloaded).  Chunk 2's v-transposes
    are deferred into B's first filler slots.
  - stage C: output projection software-pipelined INTO stage B at one
    [128,512] output tile (4 accumulating matmuls) per score slot.
    Output DMAs are paired (one [128,1024] DMA per two tiles) to halve
    sync-queue triggers.  The last 3 filler slots of B are left empty so
    those output tiles run right after the final ctx matmul, covering
    the last softmax-normalize chain (removes the B->drain bubble).
Measured ~421-428us on HW at fast clock (the PE clock varies ~10%
run-to-run with chip power state; slow-clock runs land ~500us for ANY
kernel version).  PE idle within the matmul span is ~2.9us vs ~22us
for the baseline, the matmul issue rate sits at the theoretical
216ns/N=512-matmul, and the HAM clock-gate stays warm end-to-end;
rel err ~1e-3.
"""

import os
import sys

import numpy as np

for _p in ("/opt/trn_rl_repo", "/root/.axon_site/_ro/trn_rl_repo"):
    if os.path.isdir(_p) and _p not in sys.path:
        sys.path.insert(0, _p)

import concourse.bass as bass
import concourse.bass_isa as bass_isa
import concourse.bacc as bacc
import concourse.mybir as mybir
import concourse.tile as tile
from concourse.bass_utils import run_bass_kernel_spmd
from concourse.masks import make_identity

P = 128          # partitions / head dim / PE tile
S = 2048         # sequence length
HID = 4096       # hidden dim
NCORES = 8
NH = 4           # q heads per core
DQ = NH * P      # per-core q width (512)
SC = 512         # free-dim chunk (PSUM bank = 512 fp32)
NKT = HID // P   # 32 contraction tiles over hidden
NCH = S // SC    # 4 sequence chunks
NJT = S // P     # 16 key tiles
NOC = HID // SC  # 8 out column chunks
SCALE = float(P) ** -0.5
F16 = mybir.dt.float16
F32 = mybir.dt.float32
F16NP = np.float16

_CACHE = {}


def _build():
    nc = bacc.Bacc(None, target_bir_lowering=False)
    xT = nc.declare_dram_parameter("xT", [HID, S], F16, isOutput=False)
    # weights pre-laid-out on host in SBUF-native order (contiguous DMA)
    wkd = nc.declare_dram_parameter("wkd", [P, NKT * P], F16, isOutput=False)
    wvd = nc.declare_dram_parameter("wvd", [P, NKT * P], F16, isOutput=False)
    wqd = nc.declare_dram_parameter("wqd", [P, NKT * DQ], F16, isOutput=False)
    wod = nc.declare_dram_parameter("wod", [P, NH * HID], F16, isOutput=False)
    bvp = nc.declare_dram_parameter("bvp", [P, 1], F32, isOutput=False)
    onesd = nc.declare_dram_parameter("onesd", [P, P], F16, isOutput=False)
    out = nc.declare_dram_parameter("out", [S, HID], F16, isOutput=True)

    EXP = mybir.ActivationFunctionType.Exp
    LN = mybir.ActivationFunctionType.Ln
    IDENT = mybir.ActivationFunctionType.Identity

    # x viewed as [kt-pair, p, 2, s] for 2-tile-batched ring DMAs and
    # [kt-quad, p, 4, s] for the direct chunk-3 loads
    xTr = xT[:, :].rearrange("(a t p) s -> a p t s", p=P, t=2)
    xTq = xT[:, :].rearrange("(a t p) s -> a p t s", p=P, t=4)

    with tile.TileContext(nc) as tc:
        with (
            tc.tile_pool(name="consts", bufs=1) as consts,
            tc.tile_pool(name="acts", bufs=1) as acts,
            tc.tile_pool(name="xin", bufs=7) as xin,
            tc.tile_pool(name="epool", bufs=5) as epool,
            tc.tile_pool(name="rpool", bufs=2) as rpool,
            tc.tile_pool(name="opool", bufs=4) as opool,
        ):
            # ---- constants / warmup dependencies first ----
            zwarm = consts.tile([P, SC], F16)
            nc.gpsimd.memset(zwarm, 0.0)
            # Activation table serving Exp, Ln AND Identity
            # (act_func_sets[6], "natural_log_exp_and_others").
            _tl = mybir.InstLoadActFuncSet(
                name=nc.get_next_instruction_name(), ins=[], outs=[])
            _tl.act_func_set_id = 6
            nc.scalar.add_instruction(_tl)
            nbias = consts.tile([P, 1], F32)
            nc.gpsimd.memset(nbias, -1.0)
            ident = consts.tile([P, P], F16)
            make_identity(nc, ident)

            # Weight tiles; their DMAs are interleaved with the x stream
            # on the SINGLE sync queue, in exact consumption order (a
            # queue's transfers serialize in order at ~350GB/s with 256KB
            # pieces, so order = priority and there is no cross-queue
            # bandwidth fight).  The scalar queue carries no DMA at all.
            wk = consts.tile([P, NKT, P], F16)
            wv = consts.tile([P, NKT, P], F16)
            wq = consts.tile([P, NKT, DQ], F16)
            bv_sb = consts.tile([P, 1], F32)
            onesf = consts.tile([P, P], F16)
            wo = consts.tile([P, NH, HID], F16)
            s3 = (NCH - 1) * SC
            x3 = consts.tile([P, NKT, SC], F16)

            # ---- persistent activations (fp16) ----
            # per-head q tiles: SEPARATE tiles so stage B's first score
            # matmul (head 0) doesn't wait on the other heads' PSUM
            # copy-outs at the A->B boundary (dependency tracking on one
            # big tile is coarser than per-slice)
            qTh = [acts.tile([P, S], F16, name="qTh%d" % _m)
                   for _m in range(NH)]
            kT = acts.tile([P, S], F16)         # [128 d, 2048 s]
            vT = acts.tile([P, S], F16)         # [128 d, 2048 s]
            v = acts.tile([P, NJT, P], F16)     # [128 j, jt, 128 d]
            ctxT = acts.tile([P, NH, S], F16)   # per head: [128 d, 2048 i]

            # ---- PE warmup: keep TensorE busy from ~6.6us (zwarm memset
            # is the only dependency) until the first weights land, so the
            # HAM clock-gate is released early ----
            with tc.tile_pool(name="pwarm", bufs=1, space="PSUM") as pwarm:
                wt = pwarm.tile([P, SC], F32, name="warm")
                for _ in range(6):
                    nc.tensor.matmul(wt, lhsT=zwarm[:, 0:P], rhs=zwarm,
                                     start=True, stop=True)

            # ---- stage A: projections (stream x once).  Chunk 0 lags the
            # q matmuls by LAG k-tiles so the k/v matmuls cover the wq DMA.
            # Chunk 3 skips q entirely (deferred into B's filler slots) and
            # reads x from the persistent x3 buffer. ----
            with tc.tile_pool(name="pacc", bufs=1, space="PSUM") as pacc:
                # Chunk order: 3 first (k/v only — needs just wk/wv, so it
                # runs while wq's 4MB streams), then 0,1,2 (full k/v/q).
                # All x DMAs issued up front in consumption order, 2
                # kt-tiles per trigger; the 5-buffer ring auto-throttles.
                CORDER = [3, 0, 1, 2]
                all_x = {}

                def xtrig(c, g2):
                    xt2 = all_x[(c, g2)] = xin.tile([P, 2, SC], F16,
                                                    name="xt2")
                    nc.sync.dma_start(out=xt2,
                                      in_=xTr[g2, :, :, c * SC:(c + 1) * SC])

                def wtrig(buf, src, k0, k1, width):
                    nc.sync.dma_start(out=buf[:, k0:k1, :],
                                      in_=src[:, k0 * width:k1 * width])

                # consumption-ordered single-queue DMA program.  The
                # trigger engine costs ~650ns per DMA_DIRECT2D, so the
                # startup window uses FEW, LARGE pieces; chunk 3's x goes
                # DIRECTLY into the persistent x3 (no ring, no copies).
                wtrig(wk, wkd, 0, 4, P)
                wtrig(wv, wvd, 0, 4, P)
                nc.sync.dma_start(out=x3[:, 0:4, :],
                                  in_=xTq[0, :, :, s3:s3 + SC])
                wtrig(wk, wkd, 4, 32, P)
                nc.sync.dma_start(out=x3[:, 4:8, :],
                                  in_=xTq[1, :, :, s3:s3 + SC])
                wtrig(wv, wvd, 4, 32, P)
                nc.sync.dma_start(out=x3[:, 8:12, :],
                                  in_=xTq[2, :, :, s3:s3 + SC])
                for qi in range(3, 8):
                    nc.sync.dma_start(out=x3[:, 4 * qi:4 * qi + 4, :],
                                      in_=xTq[qi, :, :, s3:s3 + SC])
                nc.sync.dma_start(out=bv_sb, in_=bvp[:, :])
                wtrig(wq, wqd, 0, 8, DQ)
                for g2 in range(0, 4):
                    xtrig(0, g2)
                wtrig(wq, wqd, 8, 16, DQ)
                for g2 in range(4, 8):
                    xtrig(0, g2)
                wtrig(wq, wqd, 16, 24, DQ)
                for g2 in range(8, 12):
                    xtrig(0, g2)
                wtrig(wq, wqd, 24, 32, DQ)
                for g2 in range(12, 16):
                    xtrig(0, g2)
                nc.sync.dma_start(out=onesf, in_=onesd[:, :])
                for g2 in range(0, 8):
                    xtrig(1, g2)
                nc.sync.dma_start(out=wo[:, 0, :], in_=wod[:, 0:HID])
                nc.sync.dma_start(out=wo[:, 1, :], in_=wod[:, HID:2 * HID])
                for g2 in range(8, 16):
                    xtrig(1, g2)
                nc.sync.dma_start(out=wo[:, 2, :], in_=wod[:, 2 * HID:3 * HID])
                nc.sync.dma_start(out=wo[:, 3, :], in_=wod[:, 3 * HID:4 * HID])
                for g2 in range(16):
                    xtrig(2, g2)

                def xtile(c, kt):
                    if c == NCH - 1:
                        return x3[:, kt, :]
                    return all_x[(c, kt // 2)][:, kt % 2, :]

                junk_ps = pacc.tile([P, SC], F32, tag="pq0", name="junk_ps")
                for c in CORDER:
                    s0 = c * SC
                    do_q = c != NCH - 1
                    last_c = c == CORDER[-1]
                    # q trails k/v, and the four heads' q streams are
                    # SKEWED by one slot each so their accumulation stops
                    # (and PSUM copy-outs, which gate the pacc->pbc pool
                    # barrier) stagger across the tail instead of all
                    # landing in the final slot.
                    lag = 0 if not do_q else (4 if last_c else 2)
                    if do_q:
                        q_ps = [pacc.tile([P, SC], F32, tag="pq%d" % m,
                                          name="q_ps%d" % m) for m in range(NH)]
                    k_ps = pacc.tile([P, SC], F32, tag="pk", bufs=2)
                    v_ps = pacc.tile([P, SC], F32, tag="pv", bufs=2)
                    nslots = NKT + (lag + NH - 1 if do_q else 0)
                    for sl in range(nslots):
                        if sl < NKT:
                            kt = sl
                            xt = xtile(c, kt)
                            st, sp = kt == 0, kt == NKT - 1
                            nc.tensor.matmul(k_ps, lhsT=wk[:, kt, :], rhs=xt,
                                             start=st, stop=sp)
                            nc.tensor.matmul(v_ps, lhsT=wv[:, kt, :], rhs=xt,
                                             start=st, stop=sp)
                            if c == NCH - 1 and kt < 20:
                                # pace the k/v-only burn while the DMA
                                # pipeline ramps; also keeps HAM warm.
                                nc.tensor.matmul(junk_ps,
                                                 lhsT=zwarm[:, 0:P],
                                                 rhs=zwarm, start=True,
                                                 stop=True)
                        if last_c and sl == NKT - 1:
                            # k/v done: copy out NOW, in parallel on Act
                            # (kT) and DVE (vT, bias deferred into B),
                            # while the skewed q tail still runs
                            nc.scalar.activation(out=kT[:, s0:s0 + SC],
                                                 in_=k_ps, func=IDENT,
                                                 scale=1.0)
                            nc.vector.tensor_copy(out=vT[:, s0:s0 + SC],
                                                  in_=v_ps)
                        if do_q:
                            for m in range(NH):
                                q_kt = sl - lag - m
                                if 0 <= q_kt < NKT:
                                    nc.tensor.matmul(
                                        q_ps[m],
                                        lhsT=wq[:, q_kt, m * P:(m + 1) * P],
                                        rhs=xtile(c, q_kt), start=q_kt == 0,
                                        stop=q_kt == NKT - 1)
                                if q_kt == NKT - 1:
                                    # head m done: copy out immediately.
                                    # Last chunk: heads 0-2 on DVE, head 3
                                    # on the idle Act engine so the pool
                                    # barrier's LAST dependency starts at
                                    # its stop instead of queuing behind
                                    # the DVE chain.  Other chunks: the
                                    # pq<m> tag ring (bufs=1) frees before
                                    # the NEXT chunk's q_ps[m] allocation.
                                    if last_c and m < 3 or not last_c and m < 2:
                                        nc.vector.tensor_copy(
                                            out=qTh[m][:, s0:s0 + SC],
                                            in_=q_ps[m])
                                    else:
                                        nc.scalar.activation(
                                            out=qTh[m][:, s0:s0 + SC],
                                            in_=q_ps[m], func=IDENT,
                                            scale=1.0)
                    # PSUM copy-out (the last chunk's were emitted
                    # inline above, staggered through the q tail).  vT is
                    # copied in [P,128] pieces so each v-transpose below
                    # starts as soon as its own slice lands instead of
                    # waiting the full 512-wide copy.
                    if not last_c:
                        nc.vector.tensor_copy(out=kT[:, s0:s0 + SC], in_=k_ps)
                        for jj in range(SC // P):
                            nc.scalar.activation(
                                out=vT[:, s0 + jj * P:s0 + (jj + 1) * P],
                                in_=v_ps[:, jj * P:(jj + 1) * P],
                                func=IDENT, bias=bv_sb, scale=1.0)
                    # v[j, d] via PE transpose; the LAST chunk's transposes
                    # are deferred into B's first filler slots.
                    if not last_c:
                        for jj in range(SC // P):
                            jt = c * (SC // P) + jj
                            t_ps = pacc.tile([P, P], F16, tag="pv", bufs=2)
                            nc.tensor.transpose(t_ps, vT[:, jt * P:(jt + 1) * P],
                                                ident)
                            nc.vector.tensor_copy(out=v[:, jt, :], in_=t_ps)

            # ---- stages B+C: attention with PE filler work software-
            # pipelined into the key-tile loop.  Per (t,h) slot the jt loop
            # yields 8 interleave positions; fillers are popped from `work`:
            # chunk-3 v-transposes (4), the deferred chunk-3 q-projection
            # (32), then output-projection tiles (128).  PSUM: pscore
            # 2x[128,1024] + pctx 2 + pout 2 = exactly 8 banks. ----
            with tc.tile_pool(name="pbc", bufs=1, space="PSUM") as pbc:
                # Fix the tag->bank mapping so stage B's first-used PSUM
                # tiles (pscore) land on the banks stage A freed EARLIEST
                # (k/v banks 4-7), and pout/pctx land on the q banks that
                # are copied out during/after the boundary.
                _db0 = pbc.tile([P, 2 * SC], F32, tag="pscore", bufs=2,
                                name="_db0")
                _db1 = pbc.tile([P, SC], F32, tag="pctx", bufs=2, name="_db1")
                work = []
                ob2_by_mt = {}

                def c_group(mt, oc, cp_eng=0, single=False):
                    m0, o0 = mt * P, oc * SC
                    o_ps = pbc.tile([P, SC], F32, tag="pout", bufs=2, name="o_ps")
                    for dt_ in range(NH):
                        nc.tensor.matmul(o_ps, lhsT=ctxT[:, dt_, m0:m0 + P],
                                         rhs=wo[:, dt_, o0:o0 + SC],
                                         start=dt_ == 0, stop=dt_ == NH - 1)
                    if oc % 2 == 0:
                        ob2 = ob2_by_mt[mt] = opool.tile([P, 2 * SC], F16,
                                                         name="ob2")
                    else:
                        ob2 = ob2_by_mt[mt]
                    half = ob2[:, (oc % 2) * SC:(oc % 2 + 1) * SC]
                    if cp_eng == 0:
                        nc.vector.tensor_copy(out=half, in_=o_ps)
                    else:
                        nc.scalar.activation(out=half, in_=o_ps, func=IDENT,
                                             scale=1.0)
                    if single:
                        nc.sync.dma_start(
                            out=out[m0:m0 + P, o0:o0 + SC], in_=half)
                    elif oc % 2 == 1:
                        nc.sync.dma_start(
                            out=out[m0:m0 + P, o0 - SC:o0 + SC], in_=ob2)

                def t_item(jt):
                    t_ps = pbc.tile([P, P], F16, tag="pscore", bufs=2,
                                    name="t_ps")
                    nc.tensor.transpose(t_ps, vT[:, jt * P:(jt + 1) * P], ident)
                    nc.vector.tensor_copy(out=v[:, jt, :], in_=t_ps)

                # Deferred chunk-3 q-projection: two passes (head pairs
                # (0,1) then (2,3)); position g covers k-tiles 2g,2g+1 for
                # both heads of the pass, reading the persistent x3.
                qstate = {}

                def q_pos(pair, g):
                    if g == 0:
                        qstate['ps'] = [
                            pbc.tile([P, SC], F32, tag="pout", bufs=2,
                                     name="q3_ps%d" % m) for m in pair]
                    for kk in (2 * g, 2 * g + 1):
                        for i, m in enumerate(pair):
                            nc.tensor.matmul(qstate['ps'][i],
                                             lhsT=wq[:, kk, m * P:(m + 1) * P],
                                             rhs=x3[:, kk, :], start=kk == 0,
                                             stop=kk == NKT - 1)
                    if g == NKT // 2 - 1:
                        for i, m in enumerate(pair):
                            nc.vector.tensor_copy(
                                out=qTh[m][:, s3:s3 + SC],
                                in_=qstate['ps'][i])

                for jt in range(8, 12):
                    work.append(("t", jt))
                for pair in ((0, 1), (2, 3)):
                    for g in range(NKT // 2):
                        work.append(("q", pair, g))

                def run_item(item, cp_eng=0, single=False):
                    if item[0] == "q":
                        q_pos(item[1], item[2])
                    elif item[0] == "t":
                        t_item(item[1])
                    else:
                        c_group(item[1], item[2], cp_eng=cp_eng,
                                single=single)

                NG = NJT // 2  # score groups of 2 key tiles
                for t in range(NCH):
                    i0 = t * SC
                    for h in range(NH):
                        last_grp = t == NCH - 1 and h == NH - 1
                        ctx_ps = pbc.tile([P, SC], F32, tag="pctx", bufs=2,
                                          name="ctx_ps")
                        racc = rpool.tile([P, SC], F16, name="racc", bufs=2)
                        e_tiles = {}
                        for g in range(NG + 2):
                            if g < NG:
                                s2 = pbc.tile([P, 2 * SC], F32, tag="pscore",
                                              bufs=2, name="s2")
                                j0 = 2 * g
                                nc.tensor.matmul(s2[:, 0:SC],
                                                 lhsT=kT[:, j0 * P:(j0 + 1) * P],
                                                 rhs=qTh[h][:, i0:i0 + SC],
                                                 start=True, stop=True)
                                nc.tensor.matmul(s2[:, SC:2 * SC],
                                                 lhsT=kT[:, (j0 + 1) * P:(j0 + 2) * P],
                                                 rhs=qTh[h][:, i0:i0 + SC],
                                                 start=True, stop=True)
                                e2 = epool.tile([P, 2 * SC], F16)
                                # exp(s*scale - 1): the -1 keeps fp16
                                # rowsums well inside range; it cancels in
                                # the softmax normalization.
                                nc.scalar.activation(out=e2, in_=s2,
                                                     func=EXP, scale=SCALE,
                                                     bias=nbias)
                                e_tiles[g] = e2
                                if t == 0 and h == 0 and g == 0:
                                    # deferred bias fix for the last A
                                    # chunk's vT (see stage-A comment);
                                    # runs before the deferred transposes
                                    # read this vT range
                                    sl = CORDER[-1] * SC
                                    nc.scalar.activation(
                                        out=vT[:, sl:sl + SC],
                                        in_=vT[:, sl:sl + SC],
                                        func=IDENT, bias=bv_sb, scale=1.0)
                            g2 = g - 2
                            if g2 >= 0:
                                e2 = e_tiles.pop(g2)
                                for half in range(2):
                                    j2 = 2 * g2 + half
                                    es = e2[:, half * SC:(half + 1) * SC]
                                    nc.tensor.matmul(ctx_ps, lhsT=v[:, j2, :],
                                                     rhs=es, start=j2 == 0,
                                                     stop=j2 == NJT - 1)
                                    if j2 == 0:
                                        nc.vector.tensor_copy(out=racc, in_=es)
                                    else:
                                        nc.vector.tensor_add(out=racc,
                                                             in0=racc, in1=es)
                                # Hold back the last 3 fillers so they run
                                # after the final ctx matmul, covering the
                                # last normalize chain (B->drain bubble).
                                if work and not (last_grp and g2 >= NG - 3):
                                    run_item(work.pop(0))
                        # cross-partition rowsum broadcast via ones-matmul,
                        # then 1/r = exp(-ln(r)) on Act (no table switch)
                        rb_ps = pbc.tile([P, SC], F32, tag="pscore", bufs=2,
                                         name="rb_ps")
                        nc.tensor.matmul(rb_ps, lhsT=onesf, rhs=racc,
                                         start=True, stop=True)
                        lnr = rpool.tile([P, SC], F32, name="lnr", bufs=2)
                        nc.scalar.activation(out=lnr, in_=rb_ps, func=LN)
                        rbc = rpool.tile([P, SC], F32, name="rbc", bufs=2)
                        nc.scalar.activation(out=rbc, in_=lnr, func=EXP,
                                             scale=-1.0)
                        nc.vector.tensor_mul(out=ctxT[:, h, i0:i0 + SC],
                                             in0=ctx_ps, in1=rbc)
                    # enqueue this chunk's output-projection tiles; they run
                    # interleaved inside B(t+1) (or in the drain loop below)
                    for mt in range(t * NCH, (t + 1) * NCH):
                        for oc in range(NOC):
                            work.append(("c", mt, oc))
                drain_i = 0
                while work:
                    # alternate drain copies DVE/Act (Act is idle here);
                    # the last two tiles DMA individually so the final
                    # transfer is 128KB, shortening the kernel tail
                    run_item(work.pop(0), cp_eng=drain_i % 2,
                             single=len(work) < 2)
                    drain_i += 1
    nc.finalize()
    return nc


def _get_program():
    if "nc" not in _CACHE:
        _CACHE["nc"] = _build()
    return _CACHE["nc"]


def _prep_inputs(hidden_states, Wq, Wk, Wv, bv, Wo):
    x = np.asarray(hidden_states, np.float32).reshape(S, HID)
    xT = np.ascontiguousarray(x.T).astype(F16NP)
    Wq = np.asarray(Wq, np.float32)
    Wk = np.asarray(Wk, np.float32)
    Wv = np.asarray(Wv, np.float32)
    bv = np.asarray(bv, np.float32)
    Wo = np.asarray(Wo, np.float32)
    maps = []
    for c in range(NCORES):
        qs = slice(c * DQ, (c + 1) * DQ)
        ks = slice(c * P, (c + 1) * P)
        # SBUF-native weight layouts: w?[p, kt, d] = W[d_out, kt*128+p]
        wkl = Wk[ks].T.reshape(NKT, P, P).transpose(1, 0, 2)
        wvl = Wv[ks].T.reshape(NKT, P, P).transpose(1, 0, 2)
        wql = Wq[qs].T.reshape(NKT, P, DQ).transpose(1, 0, 2)
        wol = Wo[:, qs].T.reshape(NH, P, HID).transpose(1, 0, 2)
        maps.append({
            "xT": xT,
            "wkd": np.ascontiguousarray(wkl.reshape(P, NKT * P)).astype(F16NP),
            "wvd": np.ascontiguousarray(wvl.reshape(P, NKT * P)).astype(F16NP),
            "wqd": np.ascontiguousarray(wql.reshape(P, NKT * DQ)).astype(F16NP),
            "wod": np.ascontiguousarray(wol.reshape(P, NH * HID)).astype(F16NP),
            "bvp": np.ascontiguousarray(bv[ks]).reshape(P, 1),
            "onesd": np.ones((P, P), F16NP),
        })
    return maps


def kernel(hidden_states, Wq, Wk, Wv, bv, Wo, _trace=False, **kw):
    nc = _get_program()
    maps = _prep_inputs(hidden_states, Wq, Wk, Wv, bv, Wo)
    res = run_bass_kernel_spmd(nc, maps, list(range(NCORES)), trace=_trace, **kw)
    out = np.zeros((S, HID), np.float32)
    for c in range(NCORES):
        out += np.asarray(res.results[c]["out"], np.float32)
    if _trace:
        return out.reshape(1, S, HID), res
    return out.reshape(1, S, HID)
